# revision 13
# baseline (speedup 1.0000x reference)
"""CBOW negative-sampling loss kernel for Trainium2 (8 NeuronCores).

Strategy: data-parallel over batch (16384 -> 8 x 2048), embedding tables
replicated per core. Per core, loop over 16 tiles of 128 batch rows:
  - all 496 int32 indices per partition are preloaded in one HWDGE DMA
    (SBUF tile [128, 16*31]; batch row t*128+p -> partition p, col t*31+s;
    slot cols 0..9 = context, 10 = center, 11..30 = negatives)
  - 31 SWDGE indirect DMAs per tile gather one embedding row per batch
    row each (10 from context_weight, 21 from center_weight) into SBUF.
    The HW indirect-DMA contract is one index per output partition, each
    descriptor moving the output's per-partition free size, so each
    gather uses a [128, 1] offset column. This SWDGE instruction stream
    (~1.27 us busy + ~0.31 us dispatch per instruction, 496 instructions)
    is the kernel's critical path and runs 100% dense; measured HW exec
    time is ~780 us/core (DMA engines ~273 us busy, DVE ~300 us, both
    hidden under the Pool stream). dma_gather (int16 indices, 256B-
    aligned rows) and TensorTensorReduce both fault on HW via this
    compile path, and multi-index indirect DMA consumes only one index
    per output partition on HW, so one-row-per-instruction indirect DMA
    is the fastest working gather primitive here.
  - DVE: strided reduce for the context sum, one broadcast multiply and
    one reduce for the 21 dot products, then a per-column +-0.1 sign/
    scale multiply (folds in the /10 context mean and the negative-score
    sign; TensorTensorReduce faults on HW via this compile path, so the
    dots use plain tensor_tensor + reduce)
  - ACT: exp(-x) then ln(1+e) with accumulate collapses the 21
    log-sigmoid terms: out[p,t] = sum_i ln(1+exp(-x_i)) = per-row loss.
    Both functions live in the natural_log_exp_and_others table set, so
    the ACT engine never swaps function tables (Softplus itself has no
    table in this compiler build; Sigmoid and Ln live in different sets
    and would force a 1.3us table swap per op)
Per-core output is [128, 16] partial losses; the host means them.
"""

import sys

for _p in ("/opt/trn_rl_repo", "/root/.axon_site/_ro/trn_rl_repo"):
    if _p not in sys.path:
        sys.path.append(_p)

import numpy as np

VOCAB = 100000
D = 300
N_CTX = 10
N_NEG = 20
N_SLOTS = 1 + N_CTX + N_NEG  # 31
N_CORES = 8
BATCH = 16384
P = 128
B_CORE = BATCH // N_CORES  # 2048
N_TILES = B_CORE // P  # 16


def emit_cbow_body(nc, tc, idx, ctx_w, cen_w, signs, out, n_tiles):
    """Emit the per-core program body into an open TileContext.

    idx:   [n_tiles*P, N_SLOTS] int32 DRAM
    ctx_w: [VOCAB, D] f32 DRAM
    cen_w: [VOCAB, D] f32 DRAM
    signs: [P, 1+N_NEG] f32 DRAM -- [+0.1, -0.1 x20] replicated rows
    out:   [P, n_tiles] f32 DRAM -- out[p, t] = sum_i ln(1+exp(-x_i))
           (= per-row loss), where x_0 is the positive score and
           x_1..x_20 the negated negative scores (all /10-scaled).
    """
    from concourse import bass, mybir

    f32 = mybir.dt.float32
    n_cn = 1 + N_NEG
    with (
        tc.tile_pool(name="gather", bufs=3) as gpool,
        tc.tile_pool(name="small", bufs=3) as spool,
        tc.tile_pool(name="accp", bufs=1) as apool,
    ):
        acc = apool.tile([P, n_tiles], f32)
        signs_sb = apool.tile([P, n_cn], f32)
        nc.sync.dma_start(out=signs_sb[:], in_=signs[:])
        # Preload every tile's indices in one DMA:
        # batch row t*P+p, slot s -> partition p, col t*N_SLOTS+s.
        idx_sb = apool.tile([P, n_tiles * N_SLOTS], mybir.dt.int32)
        nc.sync.dma_start(
            out=idx_sb[:],
            in_=idx.rearrange("(t p) s -> p t s", p=P),
        )
        for t in range(n_tiles):
            col0 = t * N_SLOTS

            ctx_embs = gpool.tile([P, N_CTX * D], f32, tag="ctx")
            for j in range(N_CTX):
                nc.gpsimd.indirect_dma_start(
                    out=ctx_embs[:, j * D : (j + 1) * D],
                    out_offset=None,
                    in_=ctx_w[:],
                    in_offset=bass.IndirectOffsetOnAxis(
                        ap=idx_sb[:, col0 + j : col0 + j + 1], axis=0
                    ),
                )
            cn_embs = gpool.tile([P, n_cn * D], f32, tag="cn")
            for j in range(n_cn):
                nc.gpsimd.indirect_dma_start(
                    out=cn_embs[:, j * D : (j + 1) * D],
                    out_offset=None,
                    in_=cen_w[:],
                    in_offset=bass.IndirectOffsetOnAxis(
                        ap=idx_sb[:, col0 + N_CTX + j : col0 + N_CTX + j + 1],
                        axis=0,
                    ),
                )

            # ctx_sum[p, d] = sum_j ctx_embs[p, j, d]  (innermost axis = j)
            ctx_sum = spool.tile([P, D], f32, tag="ctxsum")
            nc.vector.reduce_sum(
                out=ctx_sum[:],
                in_=ctx_embs.rearrange("p (j d) -> p d j", j=N_CTX),
                axis=mybir.AxisListType.X,
            )

            # prod[p, n, d] = cn_embs[p, n, d] * ctx_sum[p, d], then
            # scores[:, n] = sum_d prod[p, n, d]. Chunked over n so the
            # multiply/reduce for early slots overlaps the remaining
            # gathers and the last tile's compute tail stays short.
            prod = spool.tile([P, n_cn * D], f32, tag="prod")
            scores = spool.tile([P, n_cn], f32, tag="scores")
            for c0 in range(0, n_cn, 7):
                c1 = min(c0 + 7, n_cn)
                w = c1 - c0
                nc.vector.tensor_tensor(
                    out=prod[:, c0 * D : c1 * D].rearrange(
                        "p (n d) -> p n d", n=w
                    ),
                    in0=cn_embs[:, c0 * D : c1 * D].rearrange(
                        "p (n d) -> p n d", n=w
                    ),
                    in1=ctx_sum.unsqueeze(1).broadcast_to([P, w, D]),
                    op=mybir.AluOpType.mult,
                )
                nc.vector.reduce_sum(
                    out=scores[:, c0:c1],
                    in_=prod[:, c0 * D : c1 * D].rearrange(
                        "p (n d) -> p n d", n=w
                    ),
                    axis=mybir.AxisListType.X,
                )
            # fold in the /10 context mean and the negative-score sign
            nc.vector.tensor_tensor(
                out=scores[:],
                in0=scores[:],
                in1=signs_sb[:],
                op=mybir.AluOpType.mult,
            )

            # acc[:, t] = sum_i ln(1 + exp(-scores[:, i]))  (= row loss)
            ex = spool.tile([P, n_cn], f32, tag="ex")
            lns = spool.tile([P, n_cn], f32, tag="lns")
            nc.scalar.activation(
                out=ex[:],
                in_=scores[:],
                func=mybir.ActivationFunctionType.Exp,
                scale=-1.0,
            )
            nc.scalar.activation(
                out=lns[:],
                in_=ex[:],
                func=mybir.ActivationFunctionType.Ln,
                bias=1.0,
                accum_out=acc[:, t : t + 1],
            )
        nc.sync.dma_start(out=out[:], in_=acc[:])


def build_program(n_tiles=N_TILES, vocab=VOCAB, n_cores=N_CORES):
    from concourse import mybir
    import concourse.bacc as bacc
    import concourse.tile as tile

    nc = bacc.Bacc(
        "TRN2",
        target_bir_lowering=False,
        debug=False,
        num_devices=n_cores,
    )
    b_core = n_tiles * P
    idx = nc.dram_tensor(
        "idx", [b_core, N_SLOTS], mybir.dt.int32, kind="ExternalInput"
    ).ap()
    ctx_w = nc.dram_tensor(
        "ctx_w", [vocab, D], mybir.dt.float32, kind="ExternalInput"
    ).ap()
    cen_w = nc.dram_tensor(
        "cen_w", [vocab, D], mybir.dt.float32, kind="ExternalInput"
    ).ap()
    signs = nc.dram_tensor(
        "signs", [P, 1 + N_NEG], mybir.dt.float32, kind="ExternalInput"
    ).ap()
    out = nc.dram_tensor(
        "out", [P, n_tiles], mybir.dt.float32, kind="ExternalOutput"
    ).ap()
    with tile.TileContext(nc) as tc:
        emit_cbow_body(nc, tc, idx, ctx_w, cen_w, signs, out, n_tiles)
    nc.compile()
    return nc


_NC_CACHE = {}


def _get_program():
    if "nc" not in _NC_CACHE:
        _NC_CACHE["nc"] = build_program()
    return _NC_CACHE["nc"]


def pack_indices(context, center, negatives):
    """[BATCH, N_SLOTS] int32: ctx cols 0..9, center col 10, negs 11..30."""
    ctx = np.asarray(context, dtype=np.int32).reshape(BATCH, N_CTX)
    cen = np.asarray(center, dtype=np.int32).reshape(BATCH, 1)
    neg = np.asarray(negatives, dtype=np.int32).reshape(BATCH, N_NEG)
    return np.ascontiguousarray(np.concatenate([ctx, cen, neg], axis=1))


def make_in_maps(context, center, negatives, context_weight, center_weight):
    idx_all = pack_indices(context, center, negatives).reshape(
        N_CORES, B_CORE, N_SLOTS
    )
    w_ctx = np.ascontiguousarray(np.asarray(context_weight, dtype=np.float32))
    w_cen = np.ascontiguousarray(np.asarray(center_weight, dtype=np.float32))
    signs = np.tile(np.array([[0.1] + [-0.1] * N_NEG], dtype=np.float32), (P, 1))
    return [
        {"idx": idx_all[c], "ctx_w": w_ctx, "cen_w": w_cen, "signs": signs}
        for c in range(N_CORES)
    ]


def kernel(context, center, negatives, context_weight, center_weight):
    from concourse import bass_utils

    nc = _get_program()
    in_maps = make_in_maps(
        context, center, negatives, context_weight, center_weight
    )
    res = bass_utils.run_bass_kernel_spmd(nc, in_maps, core_ids=list(range(N_CORES)))
    acc = np.stack([r["out"] for r in res.results])  # [N_CORES, P, N_TILES]
    # acc holds per-row losses (softplus form): final = mean.
    return np.array(acc.sum(dtype=np.float64) / BATCH, dtype=np.float32)


# revision 14
# speedup vs baseline: 1.0212x; 1.0212x over previous
"""CBOW negative-sampling loss kernel for Trainium2 (8 NeuronCores).

Strategy: data-parallel over batch (16384 -> 8 x 2048), embedding tables
replicated per core. Per core, loop over 16 tiles of 128 batch rows:
  - all 496 int32 indices per partition are preloaded in one HWDGE DMA
    (SBUF tile [128, 16*31]; batch row t*128+p -> partition p, col t*31+s;
    slot cols 0..9 = context, 10 = center, 11..30 = negatives)
  - 31 SWDGE indirect DMAs per tile gather one embedding row per batch
    row each (10 from context_weight, 21 from center_weight) into SBUF.
    The HW indirect-DMA contract is one index per output partition, each
    descriptor moving the output's per-partition free size, so each
    gather uses a [128, 1] offset column. This SWDGE instruction stream
    (~1.27 us busy + ~0.31 us dispatch per instruction, 496 instructions)
    is the kernel's critical path and runs 100% dense; measured HW exec
    time is ~780 us/core (DMA engines ~273 us busy, DVE ~300 us, both
    hidden under the Pool stream). dma_gather (int16 indices, 256B-
    aligned rows) and TensorTensorReduce both fault on HW via this
    compile path, and multi-index indirect DMA consumes only one index
    per output partition on HW, so one-row-per-instruction indirect DMA
    is the fastest working gather primitive here.
  - DVE: strided reduce for the context sum, one broadcast multiply and
    one reduce for the 21 dot products, then a per-column +-0.1 sign/
    scale multiply (folds in the /10 context mean and the negative-score
    sign; TensorTensorReduce faults on HW via this compile path, so the
    dots use plain tensor_tensor + reduce)
  - ACT: exp(-x) then ln(1+e) with accumulate collapses the 21
    log-sigmoid terms: out[p,t] = sum_i ln(1+exp(-x_i)) = per-row loss.
    Both functions live in the natural_log_exp_and_others table set, so
    the ACT engine never swaps function tables (Softplus itself has no
    table in this compiler build; Sigmoid and Ln live in different sets
    and would force a 1.3us table swap per op)
Per-core output is [128, 16] partial losses; the host means them.
"""

import sys

for _p in ("/opt/trn_rl_repo", "/root/.axon_site/_ro/trn_rl_repo"):
    if _p not in sys.path:
        sys.path.append(_p)

import numpy as np

VOCAB = 100000
D = 300
N_CTX = 10
N_NEG = 20
N_SLOTS = 1 + N_CTX + N_NEG  # 31
N_CORES = 8
BATCH = 16384
P = 128
B_CORE = BATCH // N_CORES  # 2048
N_TILES = B_CORE // P  # 16


def emit_cbow_body(nc, tc, idx, ctx_w, cen_w, signs, out, n_tiles):
    """Emit the per-core program body into an open TileContext.

    idx:   [n_tiles*P, N_SLOTS] int32 DRAM
    ctx_w: [VOCAB, D] f32 DRAM
    cen_w: [VOCAB, D] f32 DRAM
    signs: [P, 1+N_NEG] f32 DRAM -- [+0.1, -0.1 x20] replicated rows
    out:   [P, n_tiles] f32 DRAM -- out[p, t] = sum_i ln(1+exp(-x_i))
           (= per-row loss), where x_0 is the positive score and
           x_1..x_20 the negated negative scores (all /10-scaled).
    """
    from concourse import bass, mybir

    f32 = mybir.dt.float32
    n_cn = 1 + N_NEG
    with (
        tc.tile_pool(name="gather", bufs=3) as gpool,
        tc.tile_pool(name="small", bufs=3) as spool,
        tc.tile_pool(name="accp", bufs=1) as apool,
    ):
        acc = apool.tile([P, n_tiles], f32)
        signs_sb = apool.tile([P, n_cn], f32)
        nc.sync.dma_start(out=signs_sb[:], in_=signs[:])
        # Preload indices: tile 0's columns first in a small DMA so its
        # gathers can issue immediately, then the rest in one bulk DMA.
        # batch row t*P+p, slot s -> partition p, col t*N_SLOTS+s.
        idx_sb = apool.tile([P, n_tiles * N_SLOTS], mybir.dt.int32)
        idx_v = idx.rearrange("(t p) s -> p t s", p=P)
        nc.sync.dma_start(out=idx_sb[:, :N_SLOTS], in_=idx_v[:, 0:1, :])
        if n_tiles > 1:
            nc.sync.dma_start(out=idx_sb[:, N_SLOTS:], in_=idx_v[:, 1:, :])
        for t in range(n_tiles):
            col0 = t * N_SLOTS

            ctx_embs = gpool.tile([P, N_CTX * D], f32, tag="ctx")
            for j in range(N_CTX):
                nc.gpsimd.indirect_dma_start(
                    out=ctx_embs[:, j * D : (j + 1) * D],
                    out_offset=None,
                    in_=ctx_w[:],
                    in_offset=bass.IndirectOffsetOnAxis(
                        ap=idx_sb[:, col0 + j : col0 + j + 1], axis=0
                    ),
                )
            cn_embs = gpool.tile([P, n_cn * D], f32, tag="cn")
            for j in range(n_cn):
                nc.gpsimd.indirect_dma_start(
                    out=cn_embs[:, j * D : (j + 1) * D],
                    out_offset=None,
                    in_=cen_w[:],
                    in_offset=bass.IndirectOffsetOnAxis(
                        ap=idx_sb[:, col0 + N_CTX + j : col0 + N_CTX + j + 1],
                        axis=0,
                    ),
                )

            # ctx_sum[p, d] = sum_j ctx_embs[p, j, d]  (innermost axis = j)
            ctx_sum = spool.tile([P, D], f32, tag="ctxsum")
            nc.vector.reduce_sum(
                out=ctx_sum[:],
                in_=ctx_embs.rearrange("p (j d) -> p d j", j=N_CTX),
                axis=mybir.AxisListType.X,
            )

            # prod[p, n, d] = cn_embs[p, n, d] * ctx_sum[p, d], then
            # scores[:, n] = sum_d prod[p, n, d]. Chunked over n so the
            # multiply/reduce for early slots overlaps the remaining
            # gathers and the last tile's compute tail stays short.
            prod = spool.tile([P, n_cn * D], f32, tag="prod")
            scores = spool.tile([P, n_cn], f32, tag="scores")
            for c0, c1 in ((0, 7), (7, 14), (14, 19), (19, n_cn)):
                w = c1 - c0
                nc.vector.tensor_tensor(
                    out=prod[:, c0 * D : c1 * D].rearrange(
                        "p (n d) -> p n d", n=w
                    ),
                    in0=cn_embs[:, c0 * D : c1 * D].rearrange(
                        "p (n d) -> p n d", n=w
                    ),
                    in1=ctx_sum.unsqueeze(1).broadcast_to([P, w, D]),
                    op=mybir.AluOpType.mult,
                )
                nc.vector.reduce_sum(
                    out=scores[:, c0:c1],
                    in_=prod[:, c0 * D : c1 * D].rearrange(
                        "p (n d) -> p n d", n=w
                    ),
                    axis=mybir.AxisListType.X,
                )
            # fold in the /10 context mean and the negative-score sign
            nc.vector.tensor_tensor(
                out=scores[:],
                in0=scores[:],
                in1=signs_sb[:],
                op=mybir.AluOpType.mult,
            )

            # acc[:, t] = sum_i ln(1 + exp(-scores[:, i]))  (= row loss)
            ex = spool.tile([P, n_cn], f32, tag="ex")
            lns = spool.tile([P, n_cn], f32, tag="lns")
            nc.scalar.activation(
                out=ex[:],
                in_=scores[:],
                func=mybir.ActivationFunctionType.Exp,
                scale=-1.0,
            )
            nc.scalar.activation(
                out=lns[:],
                in_=ex[:],
                func=mybir.ActivationFunctionType.Ln,
                bias=1.0,
                accum_out=acc[:, t : t + 1],
            )
        nc.sync.dma_start(out=out[:], in_=acc[:])


def build_program(n_tiles=N_TILES, vocab=VOCAB, n_cores=N_CORES):
    from concourse import mybir
    import concourse.bacc as bacc
    import concourse.tile as tile

    nc = bacc.Bacc(
        "TRN2",
        target_bir_lowering=False,
        debug=False,
        enable_asserts=False,
        num_devices=n_cores,
    )
    b_core = n_tiles * P
    idx = nc.dram_tensor(
        "idx", [b_core, N_SLOTS], mybir.dt.int32, kind="ExternalInput"
    ).ap()
    ctx_w = nc.dram_tensor(
        "ctx_w", [vocab, D], mybir.dt.float32, kind="ExternalInput"
    ).ap()
    cen_w = nc.dram_tensor(
        "cen_w", [vocab, D], mybir.dt.float32, kind="ExternalInput"
    ).ap()
    signs = nc.dram_tensor(
        "signs", [P, 1 + N_NEG], mybir.dt.float32, kind="ExternalInput"
    ).ap()
    out = nc.dram_tensor(
        "out", [P, n_tiles], mybir.dt.float32, kind="ExternalOutput"
    ).ap()
    with tile.TileContext(nc) as tc:
        emit_cbow_body(nc, tc, idx, ctx_w, cen_w, signs, out, n_tiles)
    nc.compile()
    return nc


_NC_CACHE = {}


def _get_program():
    if "nc" not in _NC_CACHE:
        _NC_CACHE["nc"] = build_program()
    return _NC_CACHE["nc"]


def pack_indices(context, center, negatives):
    """[BATCH, N_SLOTS] int32: ctx cols 0..9, center col 10, negs 11..30."""
    ctx = np.asarray(context, dtype=np.int32).reshape(BATCH, N_CTX)
    cen = np.asarray(center, dtype=np.int32).reshape(BATCH, 1)
    neg = np.asarray(negatives, dtype=np.int32).reshape(BATCH, N_NEG)
    return np.ascontiguousarray(np.concatenate([ctx, cen, neg], axis=1))


def make_in_maps(context, center, negatives, context_weight, center_weight):
    idx_all = pack_indices(context, center, negatives).reshape(
        N_CORES, B_CORE, N_SLOTS
    )
    w_ctx = np.ascontiguousarray(np.asarray(context_weight, dtype=np.float32))
    w_cen = np.ascontiguousarray(np.asarray(center_weight, dtype=np.float32))
    signs = np.tile(np.array([[0.1] + [-0.1] * N_NEG], dtype=np.float32), (P, 1))
    return [
        {"idx": idx_all[c], "ctx_w": w_ctx, "cen_w": w_cen, "signs": signs}
        for c in range(N_CORES)
    ]


def kernel(context, center, negatives, context_weight, center_weight):
    from concourse import bass_utils

    nc = _get_program()
    in_maps = make_in_maps(
        context, center, negatives, context_weight, center_weight
    )
    res = bass_utils.run_bass_kernel_spmd(nc, in_maps, core_ids=list(range(N_CORES)))
    acc = np.stack([r["out"] for r in res.results])  # [N_CORES, P, N_TILES]
    # acc holds per-row losses (softplus form): final = mean.
    return np.array(acc.sum(dtype=np.float64) / BATCH, dtype=np.float32)
